# revision 7
# baseline (speedup 1.0000x reference)
"""MultiHeadAttention kernel for 8x TRN2 NeuronCores.

The reference module's einsum reduces the attention tensor over BOTH the
query and key axes (attn_mass = sum_{q,k} softmax(logits)_k), and softmax
rows sum to 1, so attn_mass == Lq exactly for every (batch, head). The
whole computation therefore collapses to

    out = (Lq * (V_heads @ Wv^T + bv)).reshape(N, L, E) @ Wo^T + bo

which is a single dense GEMM after folding the (block-diagonal) per-head
V-projection into the output projection:

    out = V_flat @ W_eff + b_eff          (W_eff: 1024 x 1024)

The device kernel is the GEMM, row-sharded across 8 cores (512 rows per
core), computed in TRANSPOSED orientation: out^T[n, m] = sum_k W[k, n]
X[m, k], with fp16 operands and fp16 output (tolerance is 2e-2; fp16
keeps l2 ~5e-4).  fp16 halves HBM traffic vs fp32 AND runs the PE at
1 cycle/row, so the kernel is PE-bound at 64 x 512-row matmuls
(~13.7us warm).  Structure:

  * ALL input data rides ONE packed DRAM buffer on the sync queue,
    split into 11 large DMAs laid out in exact consumption order, so
    the first matmul's data ([X-slab0 | W-chunk(0,0)]) is the very
    first transfer served by the DMA engines and the PE's data arrival
    always leads its consumption;
  * MM order: staircase shells 0-3 (chunk (j,k) data needed grows
    ~linearly with MMs retired -> earliest possible PE start), then
    bank-sequential completion (banks 0-7 retire their k=7 chunk
    progressively from ~45% through the stream) so PSUM evictions and
    output DMAs spread across the compute instead of piling up in a
    serialized tail;
  * the engine that evicts a bank (vector: tensor_scalar_add, scalar:
    activation-Identity with per-partition bias AP, alternating)
    issues that bank's output DMA from its own HWDGE queue -- no
    cross-engine hop, no sync-sequencer serialization;
  * junk matmuls on memset data bridge the DMA latency and warm the PE
    HAM clock gate (zero data is activity-gated and does not warm it).

The host packs V-shards in transposed slab order and transposes the
(E, RPC) fp16 per-core outputs back.
"""

import numpy as np

import concourse.bass as bass
import concourse.bacc as bacc
import concourse.mybir as mybir
from concourse.tile import TileContext
from concourse.bass_utils import run_bass_kernel_spmd

N_CORES = 8
E = 1024            # embed dim == d_model
H, HD = 16, 64      # heads, head dim
ROWS = 4096         # N * L = 2 * 2048
RPC = ROWS // N_CORES   # rows per core = 512
P = 128             # SBUF partitions
KT = E // P         # 8 contraction slabs
JT = E // P         # 8 output-column banks
N_JUNK = 24         # junk fp16 matmuls bridging DMA latency + HAM warmup
N_JUNK_GAP = 5      # junk matmuls bridging the shell0 -> shell1 data gap
N_JUNK_PB = 2       # junk matmuls bridging the staircase -> phase-B boundary
SHELLS = 5          # staircase shells before bank-sequential completion

# MM emission order: staircase shells 0..SHELLS-1, then bank-sequential.
# None entries are junk matmuls keeping the PE busy (HAM warm) while the
# next transfer's completion semaphore is still in flight.
MM_ORDER = []
for s in range(SHELLS):
    for k in range(s):
        MM_ORDER.append((s, k))
    for j in range(s + 1):
        MM_ORDER.append((j, s))
    if s == 0:
        MM_ORDER.extend([None] * N_JUNK_GAP)
MM_ORDER.extend([None] * N_JUNK_PB)
for j in range(JT):
    ks = range(SHELLS, KT) if j < SHELLS else range(KT)
    for k in ks:
        MM_ORDER.append((j, k))

# Input stream: X slabs + W chunks interleaved in consumption order,
# grouped into transfers (one dma_start each, sync queue, in order).
# Entries: ("x", k) = 512 cols, ("w", j, k) = 128 cols.
TRANSFERS = []
for s in range(SHELLS):
    t = [("x", s)]
    for k in range(s):
        t.append(("w", s, k))
    for j in range(s + 1):
        t.append(("w", j, s))
    TRANSFERS.append(t)
TRANSFERS.append([("x", k) for k in range(SHELLS, KT)])
_half = (SHELLS + 1) // 2
TRANSFERS.append(
    [("w", j, k) for j in range(_half) for k in range(SHELLS, KT)]
)
TRANSFERS.append(
    [("w", j, k) for j in range(_half, SHELLS) for k in range(SHELLS, KT)]
)
for j in range(SHELLS, JT):
    TRANSFERS.append([("w", j, k) for k in range(KT)])

# column offsets in the stream buffer
X_OFF, W_OFF, T_RANGE = {}, {}, []
_off = 0
for t in TRANSFERS:
    c0 = _off
    for e in t:
        if e[0] == "x":
            X_OFF[e[1]] = _off
            _off += RPC
        else:
            W_OFF[(e[1], e[2])] = _off
            _off += P
    T_RANGE.append((c0, _off))
SCOLS = _off
assert SCOLS == KT * RPC + JT * KT * P

# eviction engine per bank: vector is faster, give it the last bank
VEC_BANKS = (0, 2, 4, 7)

_NC_CACHE = {}
LAST_RESULTS = None  # BassKernelResults of the most recent device run


def _build(dtype):
    f32 = mybir.dt.float32
    nc = bacc.Bacc(None, target_bir_lowering=False)
    stream = nc.declare_dram_parameter("stream", [P, SCOLS], dtype, isOutput=False)
    bias = nc.declare_dram_parameter("bias", [P, JT], f32, isOutput=False)
    outT = nc.declare_dram_parameter("outT", [E, RPC], dtype, isOutput=True)

    with TileContext(nc) as tc:
        with (
            tc.tile_pool(name="bp", bufs=1) as bp,
            tc.tile_pool(name="xp", bufs=1) as xp,
            tc.tile_pool(name="pp", bufs=1, space="PSUM") as pp,
            tc.tile_pool(name="op", bufs=1) as op,
        ):
            # memset needs no DMA: junk matmuls start right after the BSP
            # preamble, before any input data lands.
            wm_t = bp.tile([P, P], dtype, name="wm", tag="wm")
            nc.vector.memset(wm_t[:], 1.0)

            bias_t = bp.tile([P, JT], f32, name="bias", tag="bias")
            nc.scalar.dma_start(out=bias_t[:], in_=bias[:, :])

            stream_t = xp.tile([P, SCOLS], dtype, name="stream", tag="stream")
            for c0, c1 in T_RANGE:
                nc.sync.dma_start(
                    out=stream_t[:, c0:c1], in_=stream[:, c0:c1]
                )

            ps = [
                pp.tile([P, RPC], f32, name=f"ps{j}", tag=f"ps{j}")
                for j in range(JT)
            ]

            # PE warm-up on nonzero data starting right after the preamble,
            # so the HAM clock-gate lifts before/through the real stream.
            for i in range(N_JUNK):
                nc.tensor.matmul(
                    ps[i % JT][:, 0:P],
                    wm_t[:, :],
                    wm_t[:, :],
                    start=True,
                    stop=True,
                )

            o_t = [
                op.tile([P, RPC], dtype, name=f"o{j}", tag=f"o{j}")
                for j in range(JT)
            ]

            def evict(j):
                b = bias_t[:, j:j + 1]
                if j in VEC_BANKS:
                    # vector has no HWDGE queue; its banks' outputs ride the
                    # sync queue (idle once the input stream has issued)
                    nc.vector.tensor_scalar_add(o_t[j][:], ps[j][:], b)
                    nc.sync.dma_start(
                        out=outT[j * P:(j + 1) * P, :], in_=o_t[j][:]
                    )
                else:
                    nc.scalar.activation(
                        o_t[j][:], ps[j][:],
                        mybir.ActivationFunctionType.Identity,
                        bias=b, scale=1.0,
                    )
                    nc.scalar.dma_start(
                        out=outT[j * P:(j + 1) * P, :], in_=o_t[j][:]
                    )

            for mm in MM_ORDER:
                if mm is None:
                    # gap-filler junk MM into a bank whose real accumulation
                    # starts much later (its start=True MM clears the bank)
                    nc.tensor.matmul(
                        ps[JT - 1][:, 0:P], wm_t[:, :], wm_t[:, :],
                        start=True, stop=True,
                    )
                    continue
                j, k = mm
                nc.tensor.matmul(
                    ps[j],
                    stream_t[:, W_OFF[(j, k)]:W_OFF[(j, k)] + P],
                    stream_t[:, X_OFF[k]:X_OFF[k] + RPC],
                    start=(k == 0),
                    stop=(k == KT - 1),
                )
                if k == KT - 1:
                    evict(j)
    nc.compile()
    return nc


def _get_nc(dtype_name):
    if dtype_name not in _NC_CACHE:
        _NC_CACHE[dtype_name] = _build(getattr(mybir.dt, dtype_name))
    return _NC_CACHE[dtype_name]


def _prep_in_maps(V, Wv, bv, Wo, bo, lq, np_dt):
    V = np.asarray(V, dtype=np.float32)
    Wv64 = np.asarray(Wv, np.float64)
    Wo64 = np.asarray(Wo, np.float64)
    bv64 = np.asarray(bv, np.float64)
    bo64 = np.asarray(bo, np.float64)

    # Fold per-head V-projection + output projection + attention mass (== Lq).
    Wo_r = Wo64.reshape(E, H, HD)                       # [n, h, b]
    W_eff = lq * np.einsum("ba,nhb->han", Wv64, Wo_r, optimize=True)
    W_eff = W_eff.reshape(E, E).astype(np.float32)      # [k, n]
    b_eff = (lq * np.einsum("nhb,b->n", Wo_r, bv64) + bo64).astype(np.float32)

    # lhsT chunk (j,k)[p, c] = W_eff[k*P + p, j*P + c]
    W4 = W_eff.reshape(KT, P, JT, P).astype(np_dt)      # [k, p, j, c]
    bias_blk = np.ascontiguousarray(b_eff.reshape(JT, P).T)  # [p, j]

    # shared W regions of the stream (X regions filled per core)
    stream = np.empty((P, SCOLS), np_dt)
    for (j, k), o in W_OFF.items():
        stream[:, o:o + P] = W4[k, :, j, :]

    X = V.reshape(ROWS, E)
    in_maps = []
    for i in range(N_CORES):
        # xpk[p, k*RPC + r] = X[i*RPC + r, k*P + p]
        xpk = (
            X[i * RPC:(i + 1) * RPC, :].astype(np_dt)
            .reshape(RPC, KT, P).transpose(2, 1, 0).reshape(P, KT * RPC)
        )
        stream_i = stream.copy()
        for k, o in X_OFF.items():
            stream_i[:, o:o + RPC] = xpk[:, k * RPC:(k + 1) * RPC]
        in_maps.append({"stream": stream_i, "bias": bias_blk})
    return in_maps


def kernel(Q, K, V, Wq, bq, Wk, bk, Wv, bv, Wo, bo, dtype_name="float16", **_unused):
    global LAST_RESULTS
    if dtype_name in ("float32", "float32r"):
        dtype_name = "float16"
    n, L, e = np.asarray(V).shape
    lq = float(np.asarray(Q).shape[1])
    np_dt = np.float16 if dtype_name == "float16" else getattr(np, dtype_name, None)
    if np_dt is None:  # bfloat16 via ml_dtypes
        from ml_dtypes import bfloat16 as np_dt
    in_maps = _prep_in_maps(V, Wv, bv, Wo, bo, lq, np_dt)
    nc = _get_nc(dtype_name)
    LAST_RESULTS = run_bass_kernel_spmd(nc, in_maps, list(range(N_CORES)))
    out = np.concatenate(
        [LAST_RESULTS.results[i]["outT"].T for i in range(N_CORES)], axis=0
    ).astype(np.float32)
    return np.ascontiguousarray(out).reshape(n, L, E)


# revision 8
# speedup vs baseline: 1.1261x; 1.1261x over previous
"""MultiHeadAttention kernel for 8x TRN2 NeuronCores.

The reference module's einsum reduces the attention tensor over BOTH the
query and key axes (attn_mass = sum_{q,k} softmax(logits)_k), and softmax
rows sum to 1, so attn_mass == Lq exactly for every (batch, head). The
whole computation therefore collapses to

    out = (Lq * (V_heads @ Wv^T + bv)).reshape(N, L, E) @ Wo^T + bo

which is a single dense GEMM after folding the (block-diagonal) per-head
V-projection into the output projection:

    out = V_flat @ W_eff + b_eff          (W_eff: 1024 x 1024)

The device kernel is the GEMM, row-sharded across 8 cores (512 rows per
core), computed in TRANSPOSED orientation: out^T[n, m] = sum_k W[k, n]
X[m, k], with fp16 operands and fp16 output (tolerance is 2e-2; fp16
keeps l2 ~5e-4).  fp16 halves HBM traffic vs fp32 AND runs the PE at
1 cycle/row, so the kernel is PE-bound at 64 x 512-row matmuls
(~13.7us warm).  Structure:

  * ALL input data rides ONE packed DRAM buffer on the sync queue,
    split into 11 large DMAs laid out in exact consumption order, so
    the first matmul's data ([X-slab0 | W-chunk(0,0)]) is the very
    first transfer served by the DMA engines and the PE's data arrival
    always leads its consumption;
  * MM order: staircase shells 0-3 (chunk (j,k) data needed grows
    ~linearly with MMs retired -> earliest possible PE start), then
    bank-sequential completion (banks 0-7 retire their k=7 chunk
    progressively from ~45% through the stream) so PSUM evictions and
    output DMAs spread across the compute instead of piling up in a
    serialized tail;
  * the engine that evicts a bank (vector: tensor_scalar_add, scalar:
    activation-Identity with per-partition bias AP, alternating)
    issues that bank's output DMA from its own HWDGE queue -- no
    cross-engine hop, no sync-sequencer serialization;
  * junk matmuls on memset data bridge the DMA latency and warm the PE
    HAM clock gate (zero data is activity-gated and does not warm it).

The host packs V-shards in transposed slab order and transposes the
(E, RPC) fp16 per-core outputs back.
"""

import numpy as np

import concourse.bass as bass
import concourse.bacc as bacc
import concourse.mybir as mybir
from concourse.tile import TileContext
from concourse.bass_utils import run_bass_kernel_spmd

N_CORES = 8
E = 1024            # embed dim == d_model
H, HD = 16, 64      # heads, head dim
ROWS = 4096         # N * L = 2 * 2048
RPC = ROWS // N_CORES   # rows per core = 512
P = 128             # SBUF partitions
KT = E // P         # 8 contraction slabs
JT = E // P         # 8 output-column banks
N_JUNK = 24         # junk fp16 matmuls bridging DMA latency + HAM warmup
N_JUNK_GAP = 5      # junk matmuls bridging the shell0 -> shell1 data gap
N_JUNK_PB = 2       # junk matmuls bridging the staircase -> phase-B boundary
SHELLS = 5          # staircase shells before bank-sequential completion

# MM emission order: staircase shells 0..SHELLS-1, then bank-sequential.
# None entries are junk matmuls keeping the PE busy (HAM warm) while the
# next transfer's completion semaphore is still in flight.
MM_ORDER = []
for s in range(SHELLS):
    for k in range(s):
        MM_ORDER.append((s, k))
    for j in range(s + 1):
        MM_ORDER.append((j, s))
    if s == 0:
        MM_ORDER.extend([None] * N_JUNK_GAP)
MM_ORDER.extend([None] * N_JUNK_PB)
for j in range(JT):
    ks = range(SHELLS, KT) if j < SHELLS else range(KT)
    for k in ks:
        MM_ORDER.append((j, k))

# Input stream: X slabs + W chunks interleaved in consumption order,
# grouped into transfers (one dma_start each, sync queue, in order).
# Entries: ("x", k) = 512 cols, ("w", j, k) = 128 cols.
TRANSFERS = []
for s in range(SHELLS):
    t = [("x", s)]
    for k in range(s):
        t.append(("w", s, k))
    for j in range(s + 1):
        t.append(("w", j, s))
    TRANSFERS.append(t)
TRANSFERS.append([("x", k) for k in range(SHELLS, KT)])
_half = (SHELLS + 1) // 2
TRANSFERS.append(
    [("w", j, k) for j in range(_half) for k in range(SHELLS, KT)]
)
TRANSFERS.append(
    [("w", j, k) for j in range(_half, SHELLS) for k in range(SHELLS, KT)]
)
for j in range(SHELLS, JT):
    TRANSFERS.append([("w", j, k) for k in range(KT)])

# column offsets in the stream buffer
X_OFF, W_OFF, T_RANGE = {}, {}, []
_off = 0
for t in TRANSFERS:
    c0 = _off
    for e in t:
        if e[0] == "x":
            X_OFF[e[1]] = _off
            _off += RPC
        else:
            W_OFF[(e[1], e[2])] = _off
            _off += P
    T_RANGE.append((c0, _off))
SCOLS = _off
assert SCOLS == KT * RPC + JT * KT * P

# eviction engine per bank: vector is faster, give it the last bank
VEC_BANKS = (0, 2, 4, 7)

_NC_CACHE = {}
LAST_RESULTS = None  # BassKernelResults of the most recent device run


def _build(dtype):
    f32 = mybir.dt.float32
    nc = bacc.Bacc(None, target_bir_lowering=False)
    stream = nc.declare_dram_parameter("stream", [P, SCOLS], dtype, isOutput=False)
    bias = nc.declare_dram_parameter("bias", [P, JT], f32, isOutput=False)
    outT = nc.declare_dram_parameter("outT", [E, RPC], dtype, isOutput=True)

    with TileContext(nc) as tc:
        with (
            tc.tile_pool(name="bp", bufs=1) as bp,
            tc.tile_pool(name="xp", bufs=1) as xp,
            tc.tile_pool(name="pp", bufs=1, space="PSUM") as pp,
            tc.tile_pool(name="op", bufs=1) as op,
        ):
            # memset needs no DMA: junk matmuls start right after the BSP
            # preamble, before any input data lands.
            wm_t = bp.tile([P, P], dtype, name="wm", tag="wm")
            nc.vector.memset(wm_t[:], 1.0)

            bias_t = bp.tile([P, JT], f32, name="bias", tag="bias")
            nc.scalar.dma_start(out=bias_t[:], in_=bias[:, :])

            stream_t = xp.tile([P, SCOLS], dtype, name="stream", tag="stream")
            for c0, c1 in T_RANGE:
                nc.sync.dma_start(
                    out=stream_t[:, c0:c1], in_=stream[:, c0:c1]
                )

            ps = [
                pp.tile([P, RPC], f32, name=f"ps{j}", tag=f"ps{j}")
                for j in range(JT)
            ]

            # PE warm-up on nonzero data starting right after the preamble,
            # so the HAM clock-gate lifts before/through the real stream.
            for i in range(N_JUNK):
                nc.tensor.matmul(
                    ps[i % JT][:, 0:P],
                    wm_t[:, :],
                    wm_t[:, :],
                    start=True,
                    stop=True,
                )

            o_t = [
                op.tile([P, RPC], dtype, name=f"o{j}", tag=f"o{j}")
                for j in range(JT)
            ]

            def evict(j):
                b = bias_t[:, j:j + 1]
                if j == JT - 1:
                    # halve the final eviction so its first output DMA's
                    # issue/DGE latency overlaps the second half's eviction
                    hh = RPC // 2
                    for c in range(2):
                        sl = slice(c * hh, (c + 1) * hh)
                        nc.vector.tensor_scalar_add(o_t[j][:, sl], ps[j][:, sl], b)
                        nc.sync.dma_start(
                            out=outT[j * P:(j + 1) * P, sl], in_=o_t[j][:, sl]
                        )
                    return
                if j in VEC_BANKS:
                    # vector has no HWDGE queue; its banks' outputs ride the
                    # sync queue (idle once the input stream has issued)
                    nc.vector.tensor_scalar_add(o_t[j][:], ps[j][:], b)
                    nc.sync.dma_start(
                        out=outT[j * P:(j + 1) * P, :], in_=o_t[j][:]
                    )
                else:
                    nc.scalar.activation(
                        o_t[j][:], ps[j][:],
                        mybir.ActivationFunctionType.Identity,
                        bias=b, scale=1.0,
                    )
                    nc.scalar.dma_start(
                        out=outT[j * P:(j + 1) * P, :], in_=o_t[j][:]
                    )

            for mm in MM_ORDER:
                if mm is None:
                    # gap-filler junk MM into a bank whose real accumulation
                    # starts much later (its start=True MM clears the bank)
                    nc.tensor.matmul(
                        ps[JT - 1][:, 0:P], wm_t[:, :], wm_t[:, :],
                        start=True, stop=True,
                    )
                    continue
                j, k = mm
                nc.tensor.matmul(
                    ps[j],
                    stream_t[:, W_OFF[(j, k)]:W_OFF[(j, k)] + P],
                    stream_t[:, X_OFF[k]:X_OFF[k] + RPC],
                    start=(k == 0),
                    stop=(k == KT - 1),
                )
                if k == KT - 1:
                    evict(j)
    nc.compile()
    return nc


def _get_nc(dtype_name):
    if dtype_name not in _NC_CACHE:
        _NC_CACHE[dtype_name] = _build(getattr(mybir.dt, dtype_name))
    return _NC_CACHE[dtype_name]


def _prep_in_maps(V, Wv, bv, Wo, bo, lq, np_dt):
    V = np.asarray(V, dtype=np.float32)
    Wv64 = np.asarray(Wv, np.float64)
    Wo64 = np.asarray(Wo, np.float64)
    bv64 = np.asarray(bv, np.float64)
    bo64 = np.asarray(bo, np.float64)

    # Fold per-head V-projection + output projection + attention mass (== Lq).
    Wo_r = Wo64.reshape(E, H, HD)                       # [n, h, b]
    W_eff = lq * np.einsum("ba,nhb->han", Wv64, Wo_r, optimize=True)
    W_eff = W_eff.reshape(E, E).astype(np.float32)      # [k, n]
    b_eff = (lq * np.einsum("nhb,b->n", Wo_r, bv64) + bo64).astype(np.float32)

    # lhsT chunk (j,k)[p, c] = W_eff[k*P + p, j*P + c]
    W4 = W_eff.reshape(KT, P, JT, P).astype(np_dt)      # [k, p, j, c]
    bias_blk = np.ascontiguousarray(b_eff.reshape(JT, P).T)  # [p, j]

    # shared W regions of the stream (X regions filled per core)
    stream = np.empty((P, SCOLS), np_dt)
    for (j, k), o in W_OFF.items():
        stream[:, o:o + P] = W4[k, :, j, :]

    X = V.reshape(ROWS, E)
    in_maps = []
    for i in range(N_CORES):
        # xpk[p, k*RPC + r] = X[i*RPC + r, k*P + p]
        xpk = (
            X[i * RPC:(i + 1) * RPC, :].astype(np_dt)
            .reshape(RPC, KT, P).transpose(2, 1, 0).reshape(P, KT * RPC)
        )
        stream_i = stream.copy()
        for k, o in X_OFF.items():
            stream_i[:, o:o + RPC] = xpk[:, k * RPC:(k + 1) * RPC]
        in_maps.append({"stream": stream_i, "bias": bias_blk})
    return in_maps


def kernel(Q, K, V, Wq, bq, Wk, bk, Wv, bv, Wo, bo, dtype_name="float16", **_unused):
    global LAST_RESULTS
    if dtype_name in ("float32", "float32r"):
        dtype_name = "float16"
    n, L, e = np.asarray(V).shape
    lq = float(np.asarray(Q).shape[1])
    np_dt = np.float16 if dtype_name == "float16" else getattr(np, dtype_name, None)
    if np_dt is None:  # bfloat16 via ml_dtypes
        from ml_dtypes import bfloat16 as np_dt
    in_maps = _prep_in_maps(V, Wv, bv, Wo, bo, lq, np_dt)
    nc = _get_nc(dtype_name)
    LAST_RESULTS = run_bass_kernel_spmd(nc, in_maps, list(range(N_CORES)))
    out = np.concatenate(
        [LAST_RESULTS.results[i]["outT"].T for i in range(N_CORES)], axis=0
    ).astype(np.float32)
    return np.ascontiguousarray(out).reshape(n, L, E)


# revision 10
# speedup vs baseline: 1.1376x; 1.0102x over previous
"""MultiHeadAttention kernel for 8x TRN2 NeuronCores.

The reference module's einsum reduces the attention tensor over BOTH the
query and key axes (attn_mass = sum_{q,k} softmax(logits)_k), and softmax
rows sum to 1, so attn_mass == Lq exactly for every (batch, head). The
whole computation therefore collapses to

    out = (Lq * (V_heads @ Wv^T + bv)).reshape(N, L, E) @ Wo^T + bo

which is a single dense GEMM after folding the (block-diagonal) per-head
V-projection into the output projection:

    out = V_flat @ W_eff + b_eff          (W_eff: 1024 x 1024)

The device kernel is the GEMM, row-sharded across 8 cores (512 rows per
core), computed in TRANSPOSED orientation: out^T[n, m] = sum_k W[k, n]
X[m, k], with fp16 operands and fp16 output (tolerance is 2e-2; fp16
keeps l2 ~5e-4).  fp16 halves HBM traffic vs fp32 AND runs the PE at
1 cycle/row, so the kernel is PE-bound at 64 x 512-row matmuls
(~13.7us warm).  Structure:

  * ALL input data rides ONE packed DRAM buffer on the sync queue,
    split into 11 large DMAs laid out in exact consumption order, so
    the first matmul's data ([X-slab0 | W-chunk(0,0)]) is the very
    first transfer served by the DMA engines and the PE's data arrival
    always leads its consumption;
  * MM order: staircase shells 0-3 (chunk (j,k) data needed grows
    ~linearly with MMs retired -> earliest possible PE start), then
    bank-sequential completion (banks 0-7 retire their k=7 chunk
    progressively from ~45% through the stream) so PSUM evictions and
    output DMAs spread across the compute instead of piling up in a
    serialized tail;
  * the engine that evicts a bank (vector: tensor_scalar_add, scalar:
    activation-Identity with per-partition bias AP, alternating)
    issues that bank's output DMA from its own HWDGE queue -- no
    cross-engine hop, no sync-sequencer serialization;
  * junk matmuls on memset data bridge the DMA latency and warm the PE
    HAM clock gate (zero data is activity-gated and does not warm it).

The host packs V-shards in transposed slab order and transposes the
(E, RPC) fp16 per-core outputs back.
"""

import numpy as np

import concourse.bass as bass
import concourse.bacc as bacc
import concourse.mybir as mybir
from concourse.tile import TileContext
from concourse.bass_utils import run_bass_kernel_spmd

N_CORES = 8
E = 1024            # embed dim == d_model
H, HD = 16, 64      # heads, head dim
ROWS = 4096         # N * L = 2 * 2048
RPC = ROWS // N_CORES   # rows per core = 512
P = 128             # SBUF partitions
KT = E // P         # 8 contraction slabs
JT = E // P         # 8 output-column banks
N_JUNK = 24         # junk fp16 matmuls bridging DMA latency + HAM warmup
JUNK_GAP = {0: 5, 1: 2, 2: 3, 3: 2}   # junk bridging shell-boundary data gaps
N_JUNK_PB = 4       # junk matmuls bridging the staircase -> phase-B boundary
SHELLS = 5          # staircase shells before bank-sequential completion

# MM emission order: staircase shells 0..SHELLS-1, then bank-sequential.
# None entries are junk matmuls keeping the PE busy (HAM warm) while the
# next transfer's completion semaphore is still in flight.
MM_ORDER = []
for s in range(SHELLS):
    for k in range(s):
        MM_ORDER.append((s, k))
    for j in range(s + 1):
        MM_ORDER.append((j, s))
    MM_ORDER.extend([None] * JUNK_GAP.get(s, 0))
MM_ORDER.extend([None] * N_JUNK_PB)
for j in range(JT):
    ks = range(SHELLS, KT) if j < SHELLS else range(KT)
    for k in ks:
        MM_ORDER.append((j, k))

# Input stream: X slabs + W chunks interleaved in consumption order,
# grouped into transfers (one dma_start each, sync queue, in order).
# Entries: ("x", k) = 512 cols, ("w", j, k) = 128 cols.
TRANSFERS = []
for s in range(SHELLS):
    t = [("x", s)]
    for k in range(s):
        t.append(("w", s, k))
    for j in range(s + 1):
        t.append(("w", j, s))
    TRANSFERS.append(t)
TRANSFERS.append([("x", k) for k in range(SHELLS, KT)])
_half = (SHELLS + 1) // 2
TRANSFERS.append(
    [("w", j, k) for j in range(_half) for k in range(SHELLS, KT)]
)
TRANSFERS.append(
    [("w", j, k) for j in range(_half, SHELLS) for k in range(SHELLS, KT)]
)
for j in range(SHELLS, JT):
    TRANSFERS.append([("w", j, k) for k in range(KT)])

# column offsets in the stream buffer
X_OFF, W_OFF, T_RANGE = {}, {}, []
_off = 0
for t in TRANSFERS:
    c0 = _off
    for e in t:
        if e[0] == "x":
            X_OFF[e[1]] = _off
            _off += RPC
        else:
            W_OFF[(e[1], e[2])] = _off
            _off += P
    T_RANGE.append((c0, _off))
SCOLS = _off
assert SCOLS == KT * RPC + JT * KT * P

# eviction engine per bank: vector is faster, give it the last bank
VEC_BANKS = (0, 2, 4, 7)

_NC_CACHE = {}
LAST_RESULTS = None  # BassKernelResults of the most recent device run


def _build(dtype):
    f32 = mybir.dt.float32
    nc = bacc.Bacc(None, target_bir_lowering=False)
    stream = nc.declare_dram_parameter("stream", [P, SCOLS], dtype, isOutput=False)
    bias = nc.declare_dram_parameter("bias", [P, JT], f32, isOutput=False)
    outT = nc.declare_dram_parameter("outT", [E, RPC], dtype, isOutput=True)

    with TileContext(nc) as tc:
        with (
            tc.tile_pool(name="bp", bufs=1) as bp,
            tc.tile_pool(name="xp", bufs=1) as xp,
            tc.tile_pool(name="pp", bufs=1, space="PSUM") as pp,
            tc.tile_pool(name="op", bufs=1) as op,
        ):
            # memset needs no DMA: junk matmuls start right after the BSP
            # preamble, before any input data lands.
            wm_t = bp.tile([P, P], dtype, name="wm", tag="wm")
            nc.vector.memset(wm_t[:], 1.0)

            bias_t = bp.tile([P, JT], f32, name="bias", tag="bias")
            nc.scalar.dma_start(out=bias_t[:], in_=bias[:, :])

            stream_t = xp.tile([P, SCOLS], dtype, name="stream", tag="stream")
            for c0, c1 in T_RANGE:
                nc.sync.dma_start(
                    out=stream_t[:, c0:c1], in_=stream[:, c0:c1]
                )

            ps = [
                pp.tile([P, RPC], f32, name=f"ps{j}", tag=f"ps{j}")
                for j in range(JT)
            ]

            # PE warm-up on nonzero data starting right after the preamble,
            # so the HAM clock-gate lifts before/through the real stream.
            for i in range(N_JUNK):
                nc.tensor.matmul(
                    ps[i % JT][:, 0:P],
                    wm_t[:, :],
                    wm_t[:, :],
                    start=True,
                    stop=True,
                )

            o_t = [
                op.tile([P, RPC], dtype, name=f"o{j}", tag=f"o{j}")
                for j in range(JT)
            ]

            def evict(j):
                b = bias_t[:, j:j + 1]
                if j == JT - 1:
                    # halve the final eviction so its first output DMA's
                    # issue/DGE latency overlaps the second half's eviction
                    hh = RPC // 2
                    for c in range(2):
                        sl = slice(c * hh, (c + 1) * hh)
                        nc.vector.tensor_scalar_add(o_t[j][:, sl], ps[j][:, sl], b)
                        nc.sync.dma_start(
                            out=outT[j * P:(j + 1) * P, sl], in_=o_t[j][:, sl]
                        )
                    return
                if j in VEC_BANKS:
                    # vector has no HWDGE queue; its banks' outputs ride the
                    # sync queue (idle once the input stream has issued)
                    nc.vector.tensor_scalar_add(o_t[j][:], ps[j][:], b)
                    nc.sync.dma_start(
                        out=outT[j * P:(j + 1) * P, :], in_=o_t[j][:]
                    )
                else:
                    nc.scalar.activation(
                        o_t[j][:], ps[j][:],
                        mybir.ActivationFunctionType.Identity,
                        bias=b, scale=1.0,
                    )
                    nc.scalar.dma_start(
                        out=outT[j * P:(j + 1) * P, :], in_=o_t[j][:]
                    )

            for mm in MM_ORDER:
                if mm is None:
                    # gap-filler junk MM into a bank whose real accumulation
                    # starts much later (its start=True MM clears the bank)
                    nc.tensor.matmul(
                        ps[JT - 1][:, 0:P], wm_t[:, :], wm_t[:, :],
                        start=True, stop=True,
                    )
                    continue
                j, k = mm
                nc.tensor.matmul(
                    ps[j],
                    stream_t[:, W_OFF[(j, k)]:W_OFF[(j, k)] + P],
                    stream_t[:, X_OFF[k]:X_OFF[k] + RPC],
                    start=(k == 0),
                    stop=(k == KT - 1),
                )
                if k == KT - 1:
                    evict(j)
    nc.compile()
    return nc


def _get_nc(dtype_name):
    if dtype_name not in _NC_CACHE:
        _NC_CACHE[dtype_name] = _build(getattr(mybir.dt, dtype_name))
    return _NC_CACHE[dtype_name]


def _prep_in_maps(V, Wv, bv, Wo, bo, lq, np_dt):
    V = np.asarray(V, dtype=np.float32)
    Wv64 = np.asarray(Wv, np.float64)
    Wo64 = np.asarray(Wo, np.float64)
    bv64 = np.asarray(bv, np.float64)
    bo64 = np.asarray(bo, np.float64)

    # Fold per-head V-projection + output projection + attention mass (== Lq).
    Wo_r = Wo64.reshape(E, H, HD)                       # [n, h, b]
    W_eff = lq * np.einsum("ba,nhb->han", Wv64, Wo_r, optimize=True)
    W_eff = W_eff.reshape(E, E).astype(np.float32)      # [k, n]
    b_eff = (lq * np.einsum("nhb,b->n", Wo_r, bv64) + bo64).astype(np.float32)

    # lhsT chunk (j,k)[p, c] = W_eff[k*P + p, j*P + c]
    W4 = W_eff.reshape(KT, P, JT, P).astype(np_dt)      # [k, p, j, c]
    bias_blk = np.ascontiguousarray(b_eff.reshape(JT, P).T)  # [p, j]

    # shared W regions of the stream (X regions filled per core)
    stream = np.empty((P, SCOLS), np_dt)
    for (j, k), o in W_OFF.items():
        stream[:, o:o + P] = W4[k, :, j, :]

    X = V.reshape(ROWS, E)
    in_maps = []
    for i in range(N_CORES):
        # xpk[p, k*RPC + r] = X[i*RPC + r, k*P + p]
        xpk = (
            X[i * RPC:(i + 1) * RPC, :].astype(np_dt)
            .reshape(RPC, KT, P).transpose(2, 1, 0).reshape(P, KT * RPC)
        )
        stream_i = stream.copy()
        for k, o in X_OFF.items():
            stream_i[:, o:o + RPC] = xpk[:, k * RPC:(k + 1) * RPC]
        in_maps.append({"stream": stream_i, "bias": bias_blk})
    return in_maps


def kernel(Q, K, V, Wq, bq, Wk, bk, Wv, bv, Wo, bo, dtype_name="float16", **_unused):
    global LAST_RESULTS
    if dtype_name in ("float32", "float32r"):
        dtype_name = "float16"
    n, L, e = np.asarray(V).shape
    lq = float(np.asarray(Q).shape[1])
    np_dt = np.float16 if dtype_name == "float16" else getattr(np, dtype_name, None)
    if np_dt is None:  # bfloat16 via ml_dtypes
        from ml_dtypes import bfloat16 as np_dt
    in_maps = _prep_in_maps(V, Wv, bv, Wo, bo, lq, np_dt)
    nc = _get_nc(dtype_name)
    LAST_RESULTS = run_bass_kernel_spmd(nc, in_maps, list(range(N_CORES)))
    out = np.concatenate(
        [LAST_RESULTS.results[i]["outT"].T for i in range(N_CORES)], axis=0
    ).astype(np.float32)
    return np.ascontiguousarray(out).reshape(n, L, E)
